# revision 21
# baseline (speedup 1.0000x reference)
"""Trainium2 Bass kernel for nn_MultiHeadAttention_59227599012491.

Reference computation (per batch b):
    xf = x[b].reshape(S, 256)
    q  = softplus(xf @ Wq.T + bq);  k = softplus(xf @ Wk.T + bk)
    v  = xf @ Wv.T + bv
    out = ((q @ k.T) @ v) @ Wo.T + bo          (no softmax!)

No softmax -> associativity: out = q @ M + bo with
    G = k.T @ v   [256,256],   M = G @ Wo.T   [256,256]
so the S x S score matrix never exists. Sharding: B=4 batches x 2
query-halves -> 8 cores, no collectives (an NRT AllReduce of M was
measured at ~17 us fixed latency -- more than the whole dedup saves, so
k/v/G/M are recomputed by both cores of a pair; queries + output rows
are split).

Per-core pipeline (all matmuls fp16, PE computes out = lhsT.T @ rhs):
    kv loop (4 groups of 8 seq tiles): ps = x_tile @ [WkT|WvT]; DVE
        adds bk to the k plane, GpSimd adds bv to the v plane (psum ->
        fp16); batched ACT Exp+Ln softplus over each group's k planes;
        GT[d,e] += v_tile^T k_tile accumulated in PSUM across all 32
        tiles. One qT chunk is interleaved after each group so the ACT
        engine's softplus backlog hides under PE work.
    qT [e,s] = softplus(Wq x^T + bq): per-partition bias fused into the
        ACT Exp pass straight out of PSUM.
    M = G @ WoT (tiny), then outT [do,s] = M^T q^T + bo: transposed
        output so bo is per-partition (DVE tensor_scalar_add) and the
        fp16 DRAM dump is contiguous 2 KB runs per partition; the host
        un-transposes and casts back to fp32.

DMA: every input DMA moves 2 KB descriptors (host-packed layouts); each
DMA instruction occupies one HW queue (~60 GB/s at 2 KB descriptors),
so the load is split into pieces across three issuing engines (sync +
scalar HWDGE, gpsimd SWDGE) for queue parallelism, with the
first-needed pieces (Wkv, x cols 0:1024) split by partition halves to
land earliest. Output: 4 chunks of [128,1024] fp16, each written as two
[64,1024] pieces on alternating queues so the final chunk drains fast.

The activation-table pass is steered to `natural_log_exp_and_others`
(the only set holding Exp AND Ln) so the ACT engine loads its PWP
table exactly once.
"""

import numpy as np

S = 4096
SQ = 2048  # query rows per core
D = 256
P = 128
IT = D // P  # 2 contraction tiles over d
NS = S // P  # 32 sequence tiles
GRP = 8  # kv tiles per softplus batch
NG = NS // GRP
N_CORES = 8

_CACHE = {}


def _patched_act_tables(orig_fn):
    def patched(arch):
        tabs = orig_fn(arch)
        return {
            name: (s if name == "natural_log_exp_and_others" else set())
            for name, s in tabs.items()
        }

    return patched


def _build_nc():
    import concourse.bacc as bacc
    import concourse.mybir as mybir
    import concourse.tile as tile

    FP = mybir.dt.float32
    FR = mybir.dt.float16
    AF = mybir.ActivationFunctionType
    ADD = mybir.AluOpType.add

    nc = bacc.Bacc("TRN2", target_bir_lowering=False, debug=False, num_devices=1)

    # x pieces: [4, 128, 2048], piece it*2+half (4 KB descriptors)
    xp_d = nc.declare_dram_parameter("xp", [2 * 2, P, 2048], FR, isOutput=False)
    # all weights in one flat [128, 2048] tensor (4 KB descriptors):
    # cols it*1024 + [wkv 512 | wq 256 | wo 256]
    wpk_d = nc.declare_dram_parameter("wpk", [P, 2048], FR, isOutput=False)
    bias_d = nc.declare_dram_parameter("biasp", [P, 4], FP, isOutput=False)
    bkv_d = nc.declare_dram_parameter("bkv", [1, 2 * D], FP, isOutput=False)
    outp_d = nc.declare_dram_parameter("outp", [P, 2 * SQ], FR, isOutput=True)

    def mm(psum, lhsT, rhs, start, stop):
        nc.tensor.matmul(psum, lhsT, rhs, start=start, stop=stop)

    with tile.TileContext(nc) as tc:
        with (
            tc.tile_pool(name="w", bufs=1) as wpool,
            tc.tile_pool(name="big", bufs=1) as big,
            tc.tile_pool(name="tmp", bufs=4) as tpool,
            tc.tile_pool(name="psKV", bufs=3, space="PSUM") as psKV,
            tc.tile_pool(name="psG", bufs=1, space="PSUM") as psG,
            tc.tile_pool(name="psQ", bufs=3, space="PSUM") as psQ,
        ):
            # SBUF weight layout (flat): cols it*1024 + [wkv 512 | wq 256 | wo 256]
            w_sb = wpool.tile([P, 2 * 1024], FR, tag="w")
            xbT_sb = big.tile([P, IT, S], FR, tag="xbT")
            bias_sb = wpool.tile([P, 4], FP, tag="bias")
            bkv_bc = wpool.tile([P, 2 * D], FP, tag="bkv")
            kv_sb = big.tile([P, 2, NS, D], FR, tag="kv")
            qT_sb = big.tile([P, IT, SQ], FR, tag="qT")
            GT_sb = wpool.tile([P, IT, D], FR, tag="GT")
            M_sb = wpool.tile([P, IT, D], FR, tag="M")
            outT_sb = big.tile([P, IT, SQ], FR, tag="outT")

            # --- input DMAs: descriptors round-robin over the issuing
            # engine's 16-queue ring at ~fixed cost per descriptor, so use
            # maximal (4 KB) descriptors and both HWDGE rings + gpsimd ---
            nc.sync.dma_start(w_sb[:, :], wpk_d.ap()[:, :])
            nc.gpsimd.dma_start(
                bkv_bc[:, :], bkv_d.ap()[0:1, :].broadcast_to([P, 2 * D])
            )
            nc.gpsimd.dma_start(bias_sb[:, :], bias_d.ap()[:, :])
            nc.sync.dma_start(xbT_sb[:, 0, 0:2048], xp_d.ap()[0, :, :])
            nc.scalar.dma_start(xbT_sb[:, 1, 0:2048], xp_d.ap()[2, :, :])
            nc.scalar.dma_start(xbT_sb[:, 1, 2048:4096], xp_d.ap()[3, :, :])
            nc.sync.dma_start(xbT_sb[:, 0, 2048:4096], xp_d.ap()[1, :, :])


            # --- kv = x [WkT | WvT] + [bk | bv]; softplus k in 4-tile runs ---
            for t in range(NS):
                ts = slice(t * P, (t + 1) * P)
                ps = psKV.tile([P, 2 * D], FP, tag="psKV")
                for it in range(IT):
                    mm(ps[:, :], xbT_sb[:, it, ts], w_sb[:, it * 1024 : it * 1024 + 512], it == 0, it == IT - 1)
                nc.vector.tensor_tensor(
                    kv_sb[:, :, t, :],
                    ps[:, :].rearrange("p (j d) -> p j d", j=2),
                    bkv_bc[:, :].rearrange("p (j d) -> p j d", j=2),
                    op=ADD,
                )
                if t % 4 == 3:
                    tt = slice(t - 3, t + 1)
                    tmpk = tpool.tile([P, 4, D], FP, tag="tmpk")
                    nc.scalar.activation(tmpk[:, :, :], kv_sb[:, 0, tt, :], AF.Exp)
                    nc.scalar.activation(kv_sb[:, 0, tt, :], tmpk[:, :, :], AF.Ln, bias=1.0)

            # --- qT = softplus(Wq x^T + bq), [e, s]; bias fused in Exp ---
            for dt in range(IT):
                for blk in range(SQ // 512):
                    ss = slice(blk * 512, (blk + 1) * 512)
                    ps = psQ.tile([P, 512], FP, tag="psQ")
                    for it in range(IT):
                        mm(
                            ps[:, :],
                            w_sb[:, it * 1024 + 512 + dt * P : it * 1024 + 512 + (dt + 1) * P],
                            xbT_sb[:, it, ss],
                            it == 0,
                            it == IT - 1,
                        )
                    tmpq = tpool.tile([P, 512], FP, tag="tmpq")
                    nc.scalar.activation(
                        tmpq[:, :], ps[:, :], AF.Exp, bias=bias_sb[:, dt : dt + 1]
                    )
                    nc.scalar.activation(qT_sb[:, dt, ss], tmpq[:, :], AF.Ln, bias=1.0)

            # --- GT[d, e] = sum_s v[s, d] k[s, e] (pure PE streaming) ---
            GTps = []
            for dt in range(IT):
                gt = psG.tile([P, D], FP, tag=f"psG{dt}", name=f"GTps{dt}")
                GTps.append(gt)
            for dt in range(IT):
                vs = slice(dt * P, (dt + 1) * P)
                for t in range(NS):
                    mm(GTps[dt], kv_sb[:, 1, t, vs], kv_sb[:, 0, t, :], t == 0, t == NS - 1)
                nc.vector.tensor_copy(GT_sb[:, dt, :], GTps[dt][:, :])

            # --- M = GT^T @ WoT ---
            for et in range(IT):
                ps = psQ.tile([P, 512], FP, tag="psQ")
                for dt in range(IT):
                    mm(
                        ps[:, 0:D],
                        GT_sb[:, dt, et * P : (et + 1) * P],
                        w_sb[:, dt * 1024 + 768 : dt * 1024 + 1024],
                        dt == 0,
                        dt == IT - 1,
                    )
                nc.vector.tensor_copy(M_sb[:, et, :], ps[:, 0:D])

            # --- outT[do, s] = M^T q^T + bo (per-partition bias, fp16) ---
            for dot in range(IT):
                for blk in range(SQ // 512):
                    ss = slice(blk * 512, (blk + 1) * 512)
                    ps = psQ.tile([P, 512], FP, tag="psQ")
                    for et in range(IT):
                        mm(
                            ps[:, :],
                            M_sb[:, et, dot * P : (dot + 1) * P],
                            qT_sb[:, et, ss],
                            et == 0,
                            et == IT - 1,
                        )
                    nc.vector.tensor_scalar_add(
                        outT_sb[:, dot, ss], ps[:, :], bias_sb[:, 2 + dot : 3 + dot]
                    )
                    if blk == SQ // 512 - 1:
                        src_ap = outT_sb[:, dot, :]
                        off = dot * SQ
                        nc.sync.dma_start(
                            outp_d.ap()[0:64, off : off + SQ], src_ap[0:64, :]
                        )
                        nc.gpsimd.dma_start(
                            outp_d.ap()[64:P, off : off + SQ], src_ap[64:P, :]
                        )

    import concourse.hw_specs as hw_specs

    orig = bacc.get_activation_tables
    bacc.get_activation_tables = _patched_act_tables(hw_specs.get_activation_tables)
    try:
        nc.compile()
    finally:
        bacc.get_activation_tables = orig
    return nc


def _get_nc():
    nc = _CACHE.get("nc")
    if nc is None:
        nc = _build_nc()
        _CACHE["nc"] = nc
    return nc


def make_in_maps(x, Wq, bq, Wk, bk, Wv, bv, Wo, bo):
    B = x.shape[0]
    f16 = np.float16
    xf = np.asarray(x, dtype=np.float32).reshape(B, S, D)
    xfT = np.ascontiguousarray(xf.transpose(0, 2, 1).astype(f16))  # [B, 256, 4096]

    def pack_it(wT):  # [256, C] -> [128, 2*C] with it-blocks side by side
        C = wT.shape[1]
        return np.ascontiguousarray(
            wT.reshape(IT, P, C).transpose(1, 0, 2).reshape(P, IT * C)
        )

    wkv2 = np.hstack([np.asarray(Wk, f16).T, np.asarray(Wv, f16).T])  # [256, 512]
    wq2 = np.asarray(Wq, f16).T
    wo2 = np.asarray(Wo, f16).T
    wpk = np.hstack(
        [
            np.hstack([wkv2[it * P : (it + 1) * P], wq2[it * P : (it + 1) * P],
                       wo2[it * P : (it + 1) * P]])
            for it in range(IT)
        ]
    )  # [128, 2048]
    biasp = np.stack(
        [
            np.asarray(bq, np.float32)[0:P],
            np.asarray(bq, np.float32)[P:D],
            np.asarray(bo, np.float32)[0:P],
            np.asarray(bo, np.float32)[P:D],
        ],
        axis=1,
    )
    shared = {
        "wpk": np.ascontiguousarray(wpk),
        "biasp": np.ascontiguousarray(biasp),
        "bkv": np.concatenate(
            [np.asarray(bk, np.float32), np.asarray(bv, np.float32)]
        ).reshape(1, 2 * D),
    }
    in_maps = []
    for c in range(N_CORES):
        b, h = divmod(c, 2)
        xT = xfT[b]
        if h == 1:
            xT = np.concatenate([xT[:, SQ:], xT[:, :SQ]], axis=1)
        # pieces [it*2+half] = [128, 2048]
        xpc = np.ascontiguousarray(
            xT.reshape(IT, P, 2, 2048).transpose(0, 2, 1, 3).reshape(4, P, 2048)
        )
        in_maps.append({"xp": xpc, **shared})
    return in_maps


def assemble_out(results, x_shape):
    B, S_, H, W = x_shape
    out = np.empty((B, S_, D), np.float32)
    for c in range(N_CORES):
        b, h = divmod(c, 2)
        outp = results[c]["outp"]  # [128, 2*SQ] fp16: [p, dot*SQ + s]
        v = outp.reshape(P, IT, SQ).astype(np.float32)
        out[b, h * SQ : (h + 1) * SQ] = v.transpose(2, 1, 0).reshape(SQ, D)
    return out.reshape(B, S_, H, W)


def kernel(x, Wq, bq, Wk, bk, Wv, bv, Wo, bo, _trace=False):
    from concourse.bass_utils import run_bass_kernel_spmd

    nc = _get_nc()
    in_maps = make_in_maps(x, Wq, bq, Wk, bk, Wv, bv, Wo, bo)
    res = run_bass_kernel_spmd(nc, in_maps, list(range(N_CORES)), trace=_trace)
    out = assemble_out(res.results, x.shape)
    if _trace:
        _CACHE["last_result"] = res
    return out


# revision 22
# speedup vs baseline: 1.0246x; 1.0246x over previous
"""Trainium2 Bass kernel for nn_MultiHeadAttention_59227599012491.

Reference computation (per batch b):
    xf = x[b].reshape(S, 256)
    q  = softplus(xf @ Wq.T + bq);  k = softplus(xf @ Wk.T + bk)
    v  = xf @ Wv.T + bv
    out = ((q @ k.T) @ v) @ Wo.T + bo          (no softmax!)

No softmax -> associativity: out = q @ M + bo with
    G = k.T @ v   [256,256],   M = G @ Wo.T   [256,256]
so the S x S score matrix never exists. Sharding: B=4 batches x 2
query-halves -> 8 cores, no collectives (an NRT AllReduce of M was
measured at ~17 us fixed latency -- more than the whole dedup saves, so
k/v/G/M are recomputed by both cores of a pair; queries + output rows
are split).

Per-core pipeline (all matmuls fp16, PE computes out = lhsT.T @ rhs):
    kv loop (4 groups of 8 seq tiles): ps = x_tile @ [WkT|WvT]; DVE
        adds bk to the k plane, GpSimd adds bv to the v plane (psum ->
        fp16); batched ACT Exp+Ln softplus over each group's k planes;
        GT[d,e] += v_tile^T k_tile accumulated in PSUM across all 32
        tiles. One qT chunk is interleaved after each group so the ACT
        engine's softplus backlog hides under PE work.
    qT [e,s] = softplus(Wq x^T + bq): per-partition bias fused into the
        ACT Exp pass straight out of PSUM.
    M = G @ WoT (tiny), then outT [do,s] = M^T q^T + bo: transposed
        output so bo is per-partition (DVE tensor_scalar_add) and the
        fp16 DRAM dump is contiguous 2 KB runs per partition; the host
        un-transposes and casts back to fp32.

DMA: every input DMA moves 2 KB descriptors (host-packed layouts); each
DMA instruction occupies one HW queue (~60 GB/s at 2 KB descriptors),
so the load is split into pieces across three issuing engines (sync +
scalar HWDGE, gpsimd SWDGE) for queue parallelism, with the
first-needed pieces (Wkv, x cols 0:1024) split by partition halves to
land earliest. Output: 4 chunks of [128,1024] fp16, each written as two
[64,1024] pieces on alternating queues so the final chunk drains fast.

The activation-table pass is steered to `natural_log_exp_and_others`
(the only set holding Exp AND Ln) so the ACT engine loads its PWP
table exactly once.
"""

import numpy as np

S = 4096
SQ = 2048  # query rows per core
D = 256
P = 128
IT = D // P  # 2 contraction tiles over d
NS = S // P  # 32 sequence tiles
GRP = 8  # kv tiles per softplus batch
NG = NS // GRP
N_CORES = 8

_CACHE = {}


def _patched_act_tables(orig_fn):
    def patched(arch):
        tabs = orig_fn(arch)
        return {
            name: (s if name == "natural_log_exp_and_others" else set())
            for name, s in tabs.items()
        }

    return patched


def _build_nc():
    import concourse.bacc as bacc
    import concourse.mybir as mybir
    import concourse.tile as tile

    FP = mybir.dt.float32
    FR = mybir.dt.float16
    AF = mybir.ActivationFunctionType
    ADD = mybir.AluOpType.add

    nc = bacc.Bacc("TRN2", target_bir_lowering=False, debug=False, num_devices=1)

    # x pieces: [8, 128, 1024], piece it*4+cc (2 KB descriptors)
    xp_d = nc.declare_dram_parameter("xp", [2 * 4, P, 1024], FR, isOutput=False)
    # weights flat [128, 2048]: [wkv it0|it1 (1024) | wq it0|it1 (512) | wo it0|it1 (512)]
    wpk_d = nc.declare_dram_parameter("wpk", [P, 2048], FR, isOutput=False)
    bias_d = nc.declare_dram_parameter("biasp", [P, 4], FP, isOutput=False)
    bkv_d = nc.declare_dram_parameter("bkv", [1, 2 * D], FP, isOutput=False)
    outp_d = nc.declare_dram_parameter("outp", [P, 2 * SQ], FR, isOutput=True)

    def mm(psum, lhsT, rhs, start, stop):
        nc.tensor.matmul(psum, lhsT, rhs, start=start, stop=stop)

    with tile.TileContext(nc) as tc:
        with (
            tc.tile_pool(name="w", bufs=1) as wpool,
            tc.tile_pool(name="big", bufs=1) as big,
            tc.tile_pool(name="tmp", bufs=4) as tpool,
            tc.tile_pool(name="psKV", bufs=3, space="PSUM") as psKV,
            tc.tile_pool(name="psG", bufs=1, space="PSUM") as psG,
            tc.tile_pool(name="psQ", bufs=3, space="PSUM") as psQ,
        ):
            # SBUF weight layout (flat): cols it*1024 + [wkv 512 | wq 256 | wo 256]
            w_sb = wpool.tile([P, 2 * 1024], FR, tag="w")
            xbT_sb = big.tile([P, IT, S], FR, tag="xbT")
            bias_sb = wpool.tile([P, 4], FP, tag="bias")
            bkv_bc = wpool.tile([P, 2 * D], FP, tag="bkv")
            kv_sb = big.tile([P, 2, NS, D], FR, tag="kv")
            qT_sb = big.tile([P, IT, SQ], FR, tag="qT")
            GT_sb = wpool.tile([P, IT, D], FR, tag="GT")
            M_sb = wpool.tile([P, IT, D], FR, tag="M")
            outT_sb = big.tile([P, IT, SQ], FR, tag="outT")

            # --- input DMAs: each engine ring moves ~7 GB/s per DMA
            # engine (byte-limited); balance bytes across the three rings
            # (sync/scalar HWDGE + gpsimd SWDGE), first-needed bytes first ---
            nc.gpsimd.dma_start(w_sb[:, 0:1024], wpk_d.ap()[:, 0:1024])
            for cc in range(4):
                nc.sync.dma_start(
                    xbT_sb[:, 0, cc * 1024 : (cc + 1) * 1024], xp_d.ap()[cc, :, :]
                )
                nc.scalar.dma_start(
                    xbT_sb[:, 1, cc * 1024 : (cc + 1) * 1024], xp_d.ap()[4 + cc, :, :]
                )
            nc.gpsimd.dma_start(
                bkv_bc[:, :], bkv_d.ap()[0:1, :].broadcast_to([P, 2 * D])
            )
            nc.gpsimd.dma_start(bias_sb[:, :], bias_d.ap()[:, :])
            nc.gpsimd.dma_start(w_sb[:, 1024:2048], wpk_d.ap()[:, 1024:2048])

            # --- kv = x [WkT | WvT] + [bk | bv]; softplus k in 4-tile runs ---
            for t in range(NS):
                ts = slice(t * P, (t + 1) * P)
                ps = psKV.tile([P, 2 * D], FP, tag="psKV")
                for it in range(IT):
                    mm(ps[:, :], xbT_sb[:, it, ts], w_sb[:, it * 512 : it * 512 + 512], it == 0, it == IT - 1)
                nc.vector.tensor_tensor(
                    kv_sb[:, :, t, :],
                    ps[:, :].rearrange("p (j d) -> p j d", j=2),
                    bkv_bc[:, :].rearrange("p (j d) -> p j d", j=2),
                    op=ADD,
                )
                if t % 4 == 3:
                    tt = slice(t - 3, t + 1)
                    tmpk = tpool.tile([P, 4, D], FP, tag="tmpk")
                    nc.scalar.activation(tmpk[:, :, :], kv_sb[:, 0, tt, :], AF.Exp)
                    nc.scalar.activation(kv_sb[:, 0, tt, :], tmpk[:, :, :], AF.Ln, bias=1.0)

            # --- qT = softplus(Wq x^T + bq), [e, s]; bias fused in Exp ---
            for dt in range(IT):
                for blk in range(SQ // 512):
                    ss = slice(blk * 512, (blk + 1) * 512)
                    ps = psQ.tile([P, 512], FP, tag="psQ")
                    for it in range(IT):
                        mm(
                            ps[:, :],
                            w_sb[:, 1024 + it * 256 + dt * P : 1024 + it * 256 + (dt + 1) * P],
                            xbT_sb[:, it, ss],
                            it == 0,
                            it == IT - 1,
                        )
                    tmpq = tpool.tile([P, 512], FP, tag="tmpq")
                    nc.scalar.activation(
                        tmpq[:, :], ps[:, :], AF.Exp, bias=bias_sb[:, dt : dt + 1]
                    )
                    nc.scalar.activation(qT_sb[:, dt, ss], tmpq[:, :], AF.Ln, bias=1.0)

            # --- GT[d, e] = sum_s v[s, d] k[s, e] (pure PE streaming) ---
            GTps = []
            for dt in range(IT):
                gt = psG.tile([P, D], FP, tag=f"psG{dt}", name=f"GTps{dt}")
                GTps.append(gt)
            for dt in range(IT):
                vs = slice(dt * P, (dt + 1) * P)
                for t in range(NS):
                    mm(GTps[dt], kv_sb[:, 1, t, vs], kv_sb[:, 0, t, :], t == 0, t == NS - 1)
                nc.vector.tensor_copy(GT_sb[:, dt, :], GTps[dt][:, :])

            # --- M = GT^T @ WoT ---
            for et in range(IT):
                ps = psQ.tile([P, 512], FP, tag="psQ")
                for dt in range(IT):
                    mm(
                        ps[:, 0:D],
                        GT_sb[:, dt, et * P : (et + 1) * P],
                        w_sb[:, 1536 + dt * 256 : 1536 + (dt + 1) * 256],
                        dt == 0,
                        dt == IT - 1,
                    )
                nc.vector.tensor_copy(M_sb[:, et, :], ps[:, 0:D])

            # --- outT[do, s] = M^T q^T + bo (per-partition bias, fp16) ---
            for dot in range(IT):
                for blk in range(SQ // 512):
                    ss = slice(blk * 512, (blk + 1) * 512)
                    ps = psQ.tile([P, 512], FP, tag="psQ")
                    for et in range(IT):
                        mm(
                            ps[:, :],
                            M_sb[:, et, dot * P : (dot + 1) * P],
                            qT_sb[:, et, ss],
                            et == 0,
                            et == IT - 1,
                        )
                    nc.vector.tensor_scalar_add(
                        outT_sb[:, dot, ss], ps[:, :], bias_sb[:, 2 + dot : 3 + dot]
                    )
                    if blk == SQ // 512 - 1:
                        src_ap = outT_sb[:, dot, :]
                        off = dot * SQ
                        nc.sync.dma_start(
                            outp_d.ap()[0:64, off : off + SQ], src_ap[0:64, :]
                        )
                        nc.gpsimd.dma_start(
                            outp_d.ap()[64:P, off : off + SQ], src_ap[64:P, :]
                        )

    import concourse.hw_specs as hw_specs

    orig = bacc.get_activation_tables
    bacc.get_activation_tables = _patched_act_tables(hw_specs.get_activation_tables)
    try:
        nc.compile()
    finally:
        bacc.get_activation_tables = orig
    return nc


def _get_nc():
    nc = _CACHE.get("nc")
    if nc is None:
        nc = _build_nc()
        _CACHE["nc"] = nc
    return nc


def make_in_maps(x, Wq, bq, Wk, bk, Wv, bv, Wo, bo):
    B = x.shape[0]
    f16 = np.float16
    xf = np.asarray(x, dtype=np.float32).reshape(B, S, D)
    xfT = np.ascontiguousarray(xf.transpose(0, 2, 1).astype(f16))  # [B, 256, 4096]

    def pack_it(wT):  # [256, C] -> [128, 2*C] with it-blocks side by side
        C = wT.shape[1]
        return np.ascontiguousarray(
            wT.reshape(IT, P, C).transpose(1, 0, 2).reshape(P, IT * C)
        )

    wkv2 = np.hstack([np.asarray(Wk, f16).T, np.asarray(Wv, f16).T])  # [256, 512]
    wq2 = np.asarray(Wq, f16).T
    wo2 = np.asarray(Wo, f16).T
    wpk = np.hstack(
        [wkv2[0:P], wkv2[P:D], wq2[0:P], wq2[P:D], wo2[0:P], wo2[P:D]]
    )  # [128, 2048]
    biasp = np.stack(
        [
            np.asarray(bq, np.float32)[0:P],
            np.asarray(bq, np.float32)[P:D],
            np.asarray(bo, np.float32)[0:P],
            np.asarray(bo, np.float32)[P:D],
        ],
        axis=1,
    )
    shared = {
        "wpk": np.ascontiguousarray(wpk),
        "biasp": np.ascontiguousarray(biasp),
        "bkv": np.concatenate(
            [np.asarray(bk, np.float32), np.asarray(bv, np.float32)]
        ).reshape(1, 2 * D),
    }
    in_maps = []
    for c in range(N_CORES):
        b, h = divmod(c, 2)
        xT = xfT[b]
        if h == 1:
            xT = np.concatenate([xT[:, SQ:], xT[:, :SQ]], axis=1)
        # pieces [it*4+cc] = [128, 1024]
        xpc = np.ascontiguousarray(
            xT.reshape(IT, P, 4, 1024).transpose(0, 2, 1, 3).reshape(8, P, 1024)
        )
        in_maps.append({"xp": xpc, **shared})
    return in_maps


def assemble_out(results, x_shape):
    B, S_, H, W = x_shape
    out = np.empty((B, S_, D), np.float32)
    for c in range(N_CORES):
        b, h = divmod(c, 2)
        outp = results[c]["outp"]  # [128, 2*SQ] fp16: [p, dot*SQ + s]
        v = outp.reshape(P, IT, SQ).astype(np.float32)
        out[b, h * SQ : (h + 1) * SQ] = v.transpose(2, 1, 0).reshape(SQ, D)
    return out.reshape(B, S_, H, W)


def kernel(x, Wq, bq, Wk, bk, Wv, bv, Wo, bo, _trace=False):
    from concourse.bass_utils import run_bass_kernel_spmd

    nc = _get_nc()
    in_maps = make_in_maps(x, Wq, bq, Wk, bk, Wv, bv, Wo, bo)
    res = run_bass_kernel_spmd(nc, in_maps, list(range(N_CORES)), trace=_trace)
    out = assemble_out(res.results, x.shape)
    if _trace:
        _CACHE["last_result"] = res
    return out


# revision 24
# speedup vs baseline: 1.1055x; 1.0789x over previous
"""Trainium2 Bass kernel for nn_MultiHeadAttention_59227599012491.

Reference computation (per batch b):
    xf = x[b].reshape(S, 256)
    q  = softplus(xf @ Wq.T + bq);  k = softplus(xf @ Wk.T + bk)
    v  = xf @ Wv.T + bv
    weight = q @ k.T            (no softmax!)
    result = weight @ v
    out    = result @ Wo.T + bo

Because there is no softmax, attention is associative:
    result = (q @ k.T) @ v = q @ (k.T @ v) = q @ G,   G: [256, 256]
    out    = q @ (G @ Wo.T) + bo = q @ M + bo
so the S x S score matrix never needs to be materialized. Per-core work
drops to a handful of [*, 256] x [256, 256] matmuls; the kernel is
memory-bound on streaming x in and out once.

Sharding: B=4 batches x 2 query-halves -> 8 cores, no collectives.
(An NRT AllReduce of the tiny M matrix was measured at ~17 us fixed
rendezvous latency on this runtime -- more than the k/v/G dedup saves --
so each core recomputes k/v/G/M for its whole batch and only the
query/output rows are split across the pair.)

Layouts (PE computes out = lhsT.T @ rhs, contracting partition dim):
    xbT  [256, 4096]  x[b] transposed on host (queries first SQ cols)
    qT   [256, 2048]  lhsT = WqT tile, rhs = xbT     (softplus via ACT,
                      bias per-partition, fused into the Exp pass)
    kv   [4096, 512]  k and v fused: rhs = [WkT | WvT], one stationary
                      xbT tile per row tile serves both. +[bk|bv] via a
                      single DVE add; softplus on the k half in-place
                      (ACT Exp then Ln(1+t), batched over tile pairs)
    GT   [256, 256]   GT[d,e] = sum_s v[s,d] k[s,e]: lhsT = v t, rhs = k t
    M    [256, 256]   M[e,do] = sum_d GT[d,e] WoT[d,do]: lhsT = GT, rhs = WoT
    out  [2048, 256]  lhsT = qT tile, rhs = M        (bias via DVE add)

The tile scheduler interleaves the qT/GT/out matmuls into the DVE-paced
kv loop's PE gaps, so the PE runs at ~91% occupancy over its window;
the engines are jointly near-saturated (PE ~32 us, ACT ~29 us, DVE
~28 us busy per core) and the phase structure below measures faster
than every explicitly-interleaved variant tried.

The activation-table pass is steered to `natural_log_exp_and_others`
(the only set holding Exp AND Ln) so the ACT engine loads its PWP table
once instead of reloading per activation (24 loads ~= 30us saved).
"""

import numpy as np

S = 4096
SQ = 2048  # query rows per core
D = 256
P = 128
IT = D // P  # 2 input-dim tiles
DT = D // P  # 2 d-model tiles
NS = S // P  # 32 sequence tiles
BLK = 512  # free-dim block for qT
N_CORES = 8

MM_DTYPE_NAME = "float16"

_CACHE = {}


def _patched_act_tables(orig_fn):
    def patched(arch):
        tabs = orig_fn(arch)
        return {
            name: (s if name == "natural_log_exp_and_others" else set())
            for name, s in tabs.items()
        }

    return patched


def _build_nc():
    import concourse.bacc as bacc
    import concourse.mybir as mybir
    import concourse.tile as tile

    FP = mybir.dt.float32
    FR = getattr(mybir.dt, MM_DTYPE_NAME)
    AF = mybir.ActivationFunctionType
    ADD = mybir.AluOpType.add

    nc = bacc.Bacc("TRN2", target_bir_lowering=False, debug=False, num_devices=1)

    xbT_d = nc.declare_dram_parameter("xbT", [D, S], FR, isOutput=False)
    wqT_d = nc.declare_dram_parameter("wqT", [D, D], FR, isOutput=False)
    wkvT_d = nc.declare_dram_parameter("wkvT", [D, 2 * D], FR, isOutput=False)
    woT_d = nc.declare_dram_parameter("woT", [D, D], FR, isOutput=False)
    bq_d = nc.declare_dram_parameter("bq", [1, D], FP, isOutput=False)
    bkv_d = nc.declare_dram_parameter("bkv", [1, 2 * D], FP, isOutput=False)
    bo2_d = nc.declare_dram_parameter("bo2", [1, 2 * D], FP, isOutput=False)
    out_d = nc.declare_dram_parameter("out", [SQ, D], FP, isOutput=True)

    def mm(psum, lhsT, rhs, start, stop):
        nc.tensor.matmul(psum, lhsT, rhs, start=start, stop=stop)

    with tile.TileContext(nc) as tc:
        with (
            tc.tile_pool(name="w", bufs=1) as wpool,
            tc.tile_pool(name="big", bufs=1) as big,
            tc.tile_pool(name="tmp", bufs=4) as tpool,
            tc.tile_pool(name="ob", bufs=4) as opool,
            tc.tile_pool(name="psQ", bufs=3, space="PSUM") as psQ,
            tc.tile_pool(name="psKV", bufs=3, space="PSUM") as psKV,
            tc.tile_pool(name="psG", bufs=2, space="PSUM") as psG,
        ):
            wq_sb = wpool.tile([P, IT, D], FR, tag="wq")
            wo_sb = wpool.tile([P, IT, D], FR, tag="wo")
            wkv_sb = wpool.tile([P, IT, 2 * D], FR, tag="wkv")
            xbT_sb = big.tile([P, IT, S], FR, tag="xbT")
            bqT = wpool.tile([P, DT], FP, tag="bqT")
            bc_bkv = wpool.tile([P, 2 * D], FP, tag="bc_bkv")
            bc_bo2 = wpool.tile([P, 2 * D], FP, tag="bc_bo2")
            b_bc = {"bkv": bc_bkv, "bo2": bc_bo2}
            for it in range(IT):
                nc.sync.dma_start(wkv_sb[:, it, :], wkvT_d.ap()[it * P : (it + 1) * P, :])
            for it in range(IT):
                nc.sync.dma_start(
                    xbT_sb[:, it, 0:1024], xbT_d.ap()[it * P : (it + 1) * P, 0:1024]
                )
            nc.sync.dma_start(
                b_bc["bkv"][:, :], bkv_d.ap()[0:1, :].broadcast_to([P, 2 * D])
            )
            for it in range(IT):
                nc.sync.dma_start(
                    xbT_sb[:, it, 1024:2048], xbT_d.ap()[it * P : (it + 1) * P, 1024:2048]
                )
            for dt in range(DT):
                nc.sync.dma_start(
                    bqT[:, dt : dt + 1],
                    bq_d.ap()[0:1, dt * P : (dt + 1) * P].rearrange("a (p w) -> (a p) w", w=1),
                )
            for it in range(IT):
                nc.sync.dma_start(wq_sb[:, it, :], wqT_d.ap()[it * P : (it + 1) * P, :])
            for it in range(IT):
                nc.sync.dma_start(
                    xbT_sb[:, it, 2048:3072], xbT_d.ap()[it * P : (it + 1) * P, 2048:3072]
                )
            nc.sync.dma_start(
                b_bc["bo2"][:, :], bo2_d.ap()[0:1, :].broadcast_to([P, 2 * D])
            )
            for it in range(IT):
                nc.sync.dma_start(
                    xbT_sb[:, it, 3072:4096], xbT_d.ap()[it * P : (it + 1) * P, 3072:4096]
                )
            for it in range(IT):
                nc.sync.dma_start(wo_sb[:, it, :], woT_d.ap()[it * P : (it + 1) * P, :])

            kv_sb = big.tile([P, 2, NS, D], FR, tag="kv")
            qT_sb = big.tile([P, DT, SQ], FR, tag="qT")
            GT_sb = wpool.tile([P, DT, D], FR, tag="GT")
            M_sb = wpool.tile([P, DT, D], FR, tag="M")

            for t in range(NS):
                ts = slice(t * P, (t + 1) * P)
                ps = psKV.tile([P, 2 * D], FP, tag="psKV")
                for it in range(IT):
                    mm(ps[:, :], xbT_sb[:, it, ts], wkv_sb[:, it, :], it == 0, it == IT - 1)
                nc.vector.tensor_tensor(
                    kv_sb[:, :, t, :], ps[:, :].rearrange("p (j d) -> p j d", j=2),
                    b_bc["bkv"][:, :].rearrange("p (j d) -> p j d", j=2), op=ADD,
                )
                if t % 4 == 3:
                    tt = slice(t - 3, t + 1)
                    tmp = tpool.tile([P, 4, D], FP, tag="tmpk")
                    nc.scalar.activation(tmp[:, :, :], kv_sb[:, 0, tt, :], AF.Exp)
                    nc.scalar.activation(kv_sb[:, 0, tt, :], tmp[:, :, :], AF.Ln, bias=1.0)

            for dt in range(DT):
                ds = slice(dt * P, (dt + 1) * P)
                for blk in range(SQ // BLK):
                    ss = slice(blk * BLK, (blk + 1) * BLK)
                    ps = psQ.tile([P, BLK], FP, tag="psQ")
                    for it in range(IT):
                        mm(ps[:, :], wq_sb[:, it, ds], xbT_sb[:, it, ss], it == 0, it == IT - 1)
                    tmp = tpool.tile([P, BLK], FP, tag="tmpq")
                    nc.scalar.activation(tmp[:, :], ps[:, :], AF.Exp, bias=bqT[:, dt : dt + 1])
                    nc.scalar.activation(qT_sb[:, dt, ss], tmp[:, :], AF.Ln, bias=1.0)

            for dt in range(DT):
                vs = slice(dt * P, (dt + 1) * P)
                ps = psG.tile([P, D], FP, tag="psG")
                for t in range(NS):
                    mm(ps[:, :], kv_sb[:, 1, t, vs], kv_sb[:, 0, t, :], t == 0, t == NS - 1)
                nc.vector.tensor_copy(GT_sb[:, dt, :], ps[:, :])

            for et in range(DT):
                es = slice(et * P, (et + 1) * P)
                ps = psG.tile([P, D], FP, tag="psG")
                for dt in range(DT):
                    mm(ps[:, :], GT_sb[:, dt, es], wo_sb[:, dt, :], dt == 0, dt == DT - 1)
                nc.vector.tensor_copy(M_sb[:, et, :], ps[:, :])

            for pr in range(SQ // (2 * P)):
                ps = psQ.tile([P, 2, D], FP, tag="psQ")
                for j in range(2):
                    ss = slice((2 * pr + j) * P, (2 * pr + j + 1) * P)
                    for et in range(DT):
                        mm(ps[:, j, :], qT_sb[:, et, ss], M_sb[:, et, :], et == 0, et == DT - 1)
                ob = opool.tile([P, 2, D], FP, tag="ob")
                nc.vector.tensor_tensor(
                    ob[:, :, :], ps[:, :, :],
                    b_bc["bo2"][:, :].rearrange("p (j d) -> p j d", j=2), op=ADD,
                )
                nc.sync.dma_start(
                    out_d.ap()[2 * pr * P : (2 * pr + 2) * P, :].rearrange(
                        "(j p) d -> p j d", p=P
                    ),
                    ob[:, :, :],
                )

    import concourse.hw_specs as hw_specs

    orig = bacc.get_activation_tables
    bacc.get_activation_tables = _patched_act_tables(hw_specs.get_activation_tables)
    try:
        nc.compile()
    finally:
        bacc.get_activation_tables = orig
    return nc


def _get_nc():
    nc = _CACHE.get("nc")
    if nc is None:
        nc = _build_nc()
        _CACHE["nc"] = nc
    return nc


def make_in_maps(x, Wq, bq, Wk, bk, Wv, bv, Wo, bo):
    B = x.shape[0]
    mmnp = np.float16
    xf = np.asarray(x, dtype=np.float32).reshape(B, S, D)
    xfT = np.ascontiguousarray(xf.transpose(0, 2, 1).astype(mmnp))
    shared = {
        "wqT": np.ascontiguousarray(np.asarray(Wq, mmnp).T),
        "wkvT": np.ascontiguousarray(
            np.hstack([np.asarray(Wk, mmnp).T, np.asarray(Wv, mmnp).T])
        ),
        "woT": np.ascontiguousarray(np.asarray(Wo, mmnp).T),
        "bq": np.asarray(bq, np.float32).reshape(1, D),
        "bkv": np.concatenate(
            [np.asarray(bk, np.float32), np.asarray(bv, np.float32)]
        ).reshape(1, 2 * D),
        "bo2": np.tile(np.asarray(bo, np.float32), 2).reshape(1, 2 * D),
    }
    in_maps = []
    for c in range(N_CORES):
        b, h = divmod(c, 2)
        xT = xfT[b]
        if h == 1:
            xT = np.concatenate([xT[:, SQ:], xT[:, :SQ]], axis=1)
        in_maps.append({"xbT": np.ascontiguousarray(xT), **shared})
    return in_maps


def assemble_out(results, x_shape):
    B, S_, H, W = x_shape
    out = np.empty((B, S_, D), np.float32)
    for c in range(N_CORES):
        b, h = divmod(c, 2)
        out[b, h * SQ : (h + 1) * SQ] = results[c]["out"]
    return out.reshape(B, S_, H, W)


def kernel(x, Wq, bq, Wk, bk, Wv, bv, Wo, bo, _trace=False):
    from concourse.bass_utils import run_bass_kernel_spmd

    nc = _get_nc()
    in_maps = make_in_maps(x, Wq, bq, Wk, bk, Wv, bv, Wo, bo)
    res = run_bass_kernel_spmd(nc, in_maps, list(range(N_CORES)), trace=_trace)
    out = assemble_out(res.results, x.shape)
    if _trace:
        _CACHE["last_result"] = res
    return out
